# revision 75
# baseline (speedup 1.0000x reference)
"""SAGAN-style self-attention block on 8 trn2 NeuronCores.

Full inputs: x [8, 512, 64, 64], w_theta [64, 512], w_phi [64, 512],
w_g [256, 512], w_o [512, 256], gamma scalar.

Sharding: data-parallel over batch — one batch item per core. Each core runs
an identical Bass program over its own x[b]; weights are replicated.

Per-core math (C=512, n=H*W=4096, m=n/4=1024):
  theta = w_theta @ x            [64, 4096]
  phi   = pool2(w_phi @ x)       [64, 1024]
  g     = pool2(w_g @ x)         [256, 1024]
  S^T   = phi^T @ theta          [1024, 4096]  (scores, transposed layout)
  E     = exp(S^T)               (bf16; no max-subtraction: |S| < ~50)
  Z     = ones^T @ sum-tree(E)   (pair tree: 4 bf16 adds on gpsimd, 3 on
                                  DVE; broadcast row sums via one PE matmul)
  att   = (g @ E) / Z            [256, 4096]   (bf16 after normalize)
  out   = (gamma*w_o) @ att + x  [512, 4096]

Matmuls run as float32r (proj/scores; tf32-like rounding) or bf16
(attend/out). The residual add uses unrounded fp32 x; in the drain
(slices 6-7) half the residuals ride the PE via an identity matmul so the
psum drain becomes an ACT copy. Steady-state engine split per slice:
PE 33 matmuls, DVE 4 residual adds + recip + att-muls + upper fs-tree,
gpsimd (Pool) 4 fs pair-sums, ACT 8 exps. Stores are batched [128,4,512]
3D DMAs (HWDGE descriptor-gen is ~625ns per DMA regardless of size).
"""

import time
from contextlib import ExitStack

import numpy as np

import bass_rust
import concourse.bass as bass
import concourse.bass_isa as bass_isa
import concourse.mybir as mybir
import concourse.tile as tile
from concourse.bass_utils import run_bass_kernel_spmd
from concourse.masks import make_identity

P = 128
C = 512  # channels
C8 = 64  # theta/phi channels
C2 = 256  # g channels
N = 4096  # H*W
M = 1024  # pooled spatial
NS = 8  # n-slices
SL = 512  # n-slice width
MT = 8  # m-tiles of 128
F32 = mybir.dt.float32
F32R = mybir.dt.float32r
BF16 = mybir.dt.bfloat16
AX = mybir.AxisListType
ALU = mybir.AluOpType
ACTF = mybir.ActivationFunctionType


def _pool_view(ap):
    """[p, 512] slice of the conv output -> 5D maxpool view [p, h2, w2, dy, dx].

    Within an n-slice of 512 = 8 image rows: local n = (2*h2+dy)*64 + 2*w2+dx.
    """
    return ap.rearrange("p (h2 dy w2 dx) -> p h2 w2 dy dx", h2=4, dy=2, w2=32, dx=2)


def emit(nc, tc, ctx):
    x_f = nc.dram_tensor("x", [C, N], F32R, kind="ExternalInput")
    wproj = nc.dram_tensor("wproj", [C, 384], F32R, kind="ExternalInput")
    wo = nc.dram_tensor("wo", [C2, C], BF16, kind="ExternalInput")
    out_d = nc.dram_tensor("out", [C, N], F32, kind="ExternalOutput")

    persist = ctx.enter_context(tc.tile_pool(name="persist", bufs=1))

    # weights on the scalar (ACT) DMA queue in k-chunks so chunk 0 lands
    # early: the first projection matmul only waits on it + the first x chunk
    wpt = persist.tile([P, 4, 384], F32R, name="wpt")
    nc.scalar.dma_start(out=wpt[:, 0, :], in_=wproj[0:P, :])
    nc.scalar.dma_start(out=wpt[:, 1, :], in_=wproj[P : 2 * P, :])
    wp = [wpt[:, k, :] for k in range(4)]
    wot = persist.tile([P, 2, C], BF16, name="wot")

    ones_f = persist.tile([P, P], F32)
    nc.vector.memset(ones_f, 1.0)
    ones_b = persist.tile([P, P], BF16)
    nc.vector.memset(ones_b, 1.0)
    ident_f = persist.tile([P, P], F32)
    make_identity(nc, ident_f)
    ident = persist.tile([P, P], F32R)
    nc.vector.tensor_copy(ident, ident_f)

    # score psum pool lives across both phases (slice 0/1 scores start in
    # phase 1)
    spool = ctx.enter_context(tc.tile_pool(name="spsum", bufs=3, space="PSUM"))
    etp = ctx.enter_context(tc.tile_pool(name="et", bufs=3))
    fsp = ctx.enter_context(tc.tile_pool(name="fs", bufs=2))
    miscp = ctx.enter_context(tc.tile_pool(name="misc", bufs=2))

    # Warm-up for the otherwise-idle startup window (PE waits ~4us for the
    # first weight+x data): dummy exp preloads the ACT exp table, and a burst
    # of matmuls on constant data ramps the PE clock (HAM).
    actwarm = persist.tile([P, 1], F32)
    nc.scalar.activation(actwarm, ones_f[:, 0:1], ACTF.Exp)
    for wi in range(6):
        wt_ = spool.tile([P, P], F32, name="warm", tag="s0", bufs=1)
        nc.tensor.matmul(wt_, lhsT=ones_f, rhs=ones_f, start=True, stop=True)

    # x loads on the sync (SP) queue, slice-major. Slice 0 as 4 separate
    # chunk DMAs so the k-accumulation of the first projection starts after
    # ~256KB; slices 1-7 as one batched 3D DMA each (HWDGE descriptor-gen is
    # ~625ns per DMACopy regardless of size, so fewer+bigger wins). Tiles
    # are f32r (rounded at DMA time): they feed the projection matmuls
    # directly and the residual adds read them back via bitcast.
    xf = persist.tile([P, 4, N], F32R, name="xf")
    x3 = x_f.ap().rearrange("(k p) n -> p k n", k=4)
    for cc in range(4):
        nc.sync.dma_start(out=xf[:, cc, 0:SL], in_=x3[:, cc, 0:SL])
    nc.scalar.dma_start(
        out=wpt[:, 2:4, :],
        in_=wproj.ap().rearrange("(k p) o -> p k o", k=4)[:, 2:4, :],
    )
    for q in range(1, NS):
        nc.sync.dma_start(
            out=xf[:, :, q * SL : (q + 1) * SL],
            in_=x3[:, :, q * SL : (q + 1) * SL],
        )
    # after all of x: slice-7's arrival gates the whole attend pipeline,
    # while wot is first consumed by out(0) well into phase 2
    nc.sync.dma_start(
        out=wot, in_=wo.ap().rearrange("(ct p) c -> p ct c", ct=2)
    )
    out3 = out_d.ap().rearrange("(ot p) n -> p ot n", ot=4)

    theta = persist.tile([C8, N], F32R)
    phi = persist.tile([C8, M], F32R)
    g = [persist.tile([P, M], F32R, name=f"g{i}") for i in range(2)]
    gT = [persist.tile([P, C2], BF16, name=f"gT{mt}") for mt in range(MT)]

    ET = [[None] * MT for _ in range(NS)]
    FS1 = [[None] * 4 for _ in range(NS)]  # pair sums
    ZB = [None] * NS  # broadcast row sums
    ATT = [[None, None] for _ in range(NS)]
    score_done = [[False] * MT for _ in range(NS)]
    fs1_done = [[False] * 4 for _ in range(NS)]

    def emit_score(i, mt):
        # S^T tile [m 128, n 512] = phi_mt^T @ theta_i  (K = 64 channels)
        sp = spool.tile([P, SL], F32, name="sp", tag=f"s{mt % 3}", bufs=1)
        nc.tensor.matmul(
            sp,
            lhsT=phi[:, mt * P : (mt + 1) * P],
            rhs=theta[:, i * SL : (i + 1) * SL],
            start=True,
            stop=True,
            skip_group_check=True,
        )
        et = etp.tile([P, SL], BF16, name="et", tag=f"et{mt}")
        nc.scalar.activation(et, sp, ACTF.Exp)
        ET[i][mt] = et
        score_done[i][mt] = True

    def emit_fs1(i, j, eng=None):
        # pair sums on the Pool engine (gpsimd): it cannot touch PSUM, so
        # this SBUF-only stage is the one piece of DVE work it can take
        t = fsp.tile([P, SL], BF16, name="fs1", tag=f"f{j}")
        (eng or nc.gpsimd).tensor_add(t, ET[i][2 * j], ET[i][2 * j + 1])
        FS1[i][j] = t
        fs1_done[i][j] = True

    def emit_fs_rest(i, eng=None):
        # finish the pair-sum tree and hand the total to the Pool engine for
        # the cross-partition broadcast sum
        for j in range(4):
            if not fs1_done[i][j]:
                emit_fs1(i, j, eng)
        h0 = fsp.tile([P, SL], BF16, name="fs2", tag="h0")
        h1 = fsp.tile([P, SL], BF16, name="fs2", tag="h1")
        nc.vector.tensor_add(h0, FS1[i][0], FS1[i][1])
        nc.vector.tensor_add(h1, FS1[i][2], FS1[i][3])
        hh = fsp.tile([P, SL], BF16, name="fs3", tag="hh")
        nc.vector.tensor_add(hh, h0, h1)
        # broadcast row sums via ones-matmul (cross-partition reduce); rides
        # a score psum bank that is idle at this point of the iteration
        zb = spool.tile([P, SL], F32, name="zb", tag="s2", bufs=1)
        nc.tensor.matmul(
            zb, lhsT=ones_b, rhs=hh, start=True, stop=True,
            skip_group_check=True,
        )
        ZB[i] = zb

    def emit_scores_full(i):
        for mt in range(MT):
            if not score_done[i][mt]:
                emit_score(i, mt)

    def emit_ap(qp, i, lo, w, mts=range(MT), ap=None, tags=("a0", "a1"), pool=None):
        esl = slice(lo, lo + w)
        if ap is None:
            ap = [
                (pool or qp).tile([P, w], F32, name="ap", tag=tags[ct], bufs=1)
                for ct in range(2)
            ]
        for mt in mts:
            st, sp_ = (mt == 0), (mt == MT - 1)
            for ct in range(2):
                nc.tensor.matmul(
                    ap[ct],
                    lhsT=gT[mt][:, ct * P : (ct + 1) * P],
                    rhs=ET[i][mt][:, esl],
                    start=st,
                    stop=sp_,
                    skip_group_check=True,
                )
        return ap

    def emit_norm(i, ap, lo, w):
        esl = slice(lo, lo + w)
        rinv = miscp.tile([P, w], F32, name="rinv", tag="rinv")
        nc.vector.reciprocal(rinv, ZB[i][:, esl])
        att = [None, None]
        # ct0 last: the out matmuls consume ct0 first, so it must be the one
        # with the freshest margin ahead of the next iteration's out stage
        for ct in (1, 0):
            t = miscp.tile([P, w], BF16, name="att", tag=f"att{ct}")
            nc.vector.tensor_mul(t, ap[ct], rinv)
            att[ct] = t
        ATT[i] = att

    def emit_out(qp, i, lo, w, ots, obt=None, queue=None, store_split=False,
                 res_pe=False):
        # project + residual for columns [i*SL+lo, ..+w); the batched store
        # goes out once ot 3 is in (or per ot-pair when store_split)
        nsl = slice(i * SL + lo, i * SL + lo + w)
        att = ATT[i]
        if obt is None:
            # 4-deep: a slice's batched store completes ~1.5 iterations after
            # its compute, and the drain chunks must not wait on it
            tag = "ob" if w == SL else "obc"
            obt = miscp.tile([P, 4, w], F32, name="ob", tag=tag, bufs=4)
        for ot in ots:
            op_ = qp.tile([P, w], F32, name="op", tag="o", bufs=3)
            for ct in range(2):
                nc.tensor.matmul(
                    op_,
                    lhsT=wot[:, ct, ot * P : (ot + 1) * P],
                    rhs=att[ct],
                    start=(ct == 0),
                    stop=(ct == 1) and not res_pe,
                    skip_group_check=True,
                )
            if res_pe and ot >= 2:
                # drain mode: accumulate the residual on the PE via identity
                # so the psum drain is a copy on the otherwise-idle ACT
                # engine; ot 2/3 stay as DVE adds (also idle at the drain) so
                # the two psum drains run in parallel
                nc.tensor.matmul(
                    op_, lhsT=ident, rhs=xf[:, ot, nsl], start=False,
                    stop=True, skip_group_check=True,
                )
                nc.scalar.copy(out=obt[:, ot, :], in_=op_)
            else:
                # residual adds read PSUM, which gpsimd cannot touch -> DVE
                nc.vector.tensor_add(
                    obt[:, ot, :], op_, xf[:, ot, nsl].bitcast(F32)
                )
            if store_split and ot == 1:
                nc.sync.dma_start(out=out3[:, 0:2, nsl], in_=obt[:, 0:2, :])
            if store_split and ot == 2 and res_pe:
                nc.scalar.dma_start(out=out3[:, 2:3, nsl], in_=obt[:, 2:3, :])
        if 3 in ots:
            if store_split and res_pe:
                nc.sync.dma_start(out=out3[:, 3:4, nsl], in_=obt[:, 3:4, :])
            elif store_split:
                nc.scalar.dma_start(out=out3[:, 2:4, nsl], in_=obt[:, 2:4, :])
            else:
                (queue or nc.sync).dma_start(out=out3[:, :, nsl], in_=obt)
        return obt

    # phase-1 score pull-in: fill PE gaps left by the x-DMA cadence with
    # slice-0/1 score matmuls (their exps + pair sums trail on ACT/DVE)
    pull = [(0, 0), (0, 1), (1, 0)]
    for mt in range(1, 7):
        pull += [(2, mt - 1), (1, mt), (0, mt + 1)]
    pull += [(1, 6), (2, 6)]
    pulled = 0

    def pump_scores(ns, budget):
        nonlocal pulled
        done = 0
        while pulled < len(pull) and done < budget:
            i, mt = pull[pulled]
            if i > ns - 1 or mt > ns - 1:
                break
            emit_score(i, mt)
            if mt % 2 == 1:
                emit_fs1(i, mt // 2)
            pulled += 1
            done += 1

    # ---- phase 1: projections + pooling + g transposes -----------------
    with tc.tile_pool(name="ppsum", bufs=1, space="PSUM") as pp, tc.tile_pool(
        name="tpsum", bufs=1, space="PSUM"
    ) as tp:
        for ns in range(NS):
            nsl = slice(ns * SL, (ns + 1) * SL)
            msl = slice(ns * P, (ns + 1) * P)
            xr = [xf[:, k, nsl] for k in range(4)]
            ps = [
                pp.tile([P, SL], F32, name="pp", tag=f"pp{mt}", bufs=1)
                for mt in range(3)
            ]
            # g-first matmul order: their psums are ready first and gate this
            # slice's transposes
            mt_order = (1, 2, 0)
            if ns == 0:
                # slice 0: k-major so each weight/x chunk arrival during the
                # startup stream unlocks three matmuls instead of one
                for k in range(4):
                    for mt in mt_order:
                        nc.tensor.matmul(
                            ps[mt],
                            lhsT=wp[k][:, mt * P : (mt + 1) * P],
                            rhs=xr[k],
                            start=(k == 0),
                            stop=(k == 3),
                            skip_group_check=True,
                        )
            else:
                for mt in mt_order:
                    for k in range(4):
                        nc.tensor.matmul(
                            ps[mt],
                            lhsT=wp[k][:, mt * P : (mt + 1) * P],
                            rhs=xr[k],
                            start=(k == 0),
                            stop=(k == 3),
                        )
            for i in range(2):
                nc.vector.tensor_reduce(
                    out=g[i][:, msl],
                    in_=_pool_view(ps[1 + i]),
                    axis=AX.XY,
                    op=ALU.max,
                )
            nc.vector.tensor_reduce(
                out=phi[:, msl],
                in_=_pool_view(ps[0][C8:P, :]),
                axis=AX.XY,
                op=ALU.max,
            )
            # last slice: keep ACT free so the entry exps (which gate the
            # first attends) start immediately after the final score matmuls
            cp = nc.vector.tensor_copy if ns == NS - 1 else nc.scalar.copy
            cp(theta[:, nsl], ps[0][0:C8, :])
            # transpose this slice's pooled g columns into gT[ns]
            for i in range(2):
                t = tp.tile([P, P], F32R, name="tp", tag="tp")
                nc.tensor.transpose(t, g[i][:, msl], ident)
                cp(gT[ns][:, i * P : (i + 1) * P], t)
            pump_scores(ns, 3)

    # ---- phase 2: softmax / attend / project ---------------------------
    with tc.tile_pool(name="qpsum", bufs=1, space="PSUM") as qp:
        # slice-0 leftovers first: ap(0) mt7 is the earliest consumer of the
        # entry exp backlog on ACT
        emit_scores_full(0)
        emit_fs_rest(0, eng=nc.vector)
        emit_scores_full(1)
        emit_scores_full(2)
        for i in range(NS):
            last = i == NS - 1
            # out(i-1) ot3 reuses ot0's psum bank, which frees only after the
            # DVE residual add drains (~1.6us); interleave half of ap(i) so
            # the PE never waits on that ring. out(NS-2) was already flushed
            # at the end of the previous iteration.
            if 1 <= i < NS - 1:
                obt = emit_out(qp, i - 1, 0, SL, ots=(0, 1, 2))
            if not last:
                ap = emit_ap(qp, i, 0, SL, mts=range(4))
                if i >= 1:
                    emit_out(qp, i - 1, 0, SL, ots=(3,), obt=obt)
                emit_ap(qp, i, 0, SL, mts=range(4, MT), ap=ap)
                # DVE order matters: recip+att-muls for slice i must precede
                # the fs tree of slice i+1, or out(i) stalls behind it
                emit_norm(i, ap, 0, SL)
                if i + 2 < NS:
                    emit_scores_full(i + 2)
                emit_fs_rest(i + 1)
                if i == NS - 2:
                    # no scores left to overlap: flush out(6) now so its
                    # store clears the DMA engines before the drain chunks
                    obt = emit_out(
                        qp, i, 0, SL, ots=(0, 1, 2), store_split=True,
                        res_pe=True,
                    )
                    emit_out(qp, i, 0, SL, ots=(3,), obt=obt,
                             store_split=True, res_pe=True)
            else:
                # drain: narrow trailing chunks, each chunk's ap matmuls
                # emitted ahead of the previous chunk's dependent out stage
                # (chunk 1 borrows the idle score psum banks)
                chunks = ((0, 256), (256, 128), (384, 128))
                aps = [None] * 3
                aps[0] = emit_ap(qp, i, *chunks[0])
                aps[1] = emit_ap(qp, i, *chunks[1], tags=("s0", "s1"), pool=spool)
                emit_norm(i, aps[0], *chunks[0])
                emit_out(qp, i, *chunks[0], ots=(0, 1, 2, 3), res_pe=True)
                aps[2] = emit_ap(qp, i, *chunks[2])
                att1 = None
                emit_norm(i, aps[1], *chunks[1])
                att1 = ATT[i]
                # norm(c2) ahead of c1's residual adds on the in-order DVE
                # queue, so the final out matmuls are not stuck behind them
                emit_norm(i, aps[2], *chunks[2])
                att2 = ATT[i]
                ATT[i] = att1
                emit_out(qp, i, *chunks[1], ots=(0, 1, 2, 3), queue=nc.scalar,
                         res_pe=True)
                ATT[i] = att2
                emit_out(qp, i, *chunks[2], ots=(0, 1, 2, 3), res_pe=True)


def build_nc():
    nc = bass.Bass(target_bir_lowering=False, trn_type="TRN2")
    with tile.TileContext(nc) as tc:
        with ExitStack() as ctx:
            emit(nc, tc, ctx)
    bass_rust.generate_event_semaphores(nc)
    return nc


def kernel(x, w_theta, w_phi, w_g, w_o, gamma):
    import ml_dtypes

    x = np.asarray(x, dtype=np.float32)
    B = x.shape[0]
    wproj = np.ascontiguousarray(
        np.concatenate(
            [np.asarray(w_theta).T, np.asarray(w_phi).T, np.asarray(w_g).T], axis=1
        ),
        dtype=np.float32,
    )
    wo_t = np.ascontiguousarray(
        (np.float32(gamma) * np.asarray(w_o)).T.astype(ml_dtypes.bfloat16)
    )

    nc = build_nc()
    in_maps = []
    for b in range(B):
        xb = np.ascontiguousarray(x[b].reshape(C, N))
        in_maps.append({"x": xb, "wproj": wproj, "wo": wo_t})
    # retry: rare transient NRT_EXEC_UNIT_UNRECOVERABLE from stale device
    # state clears on re-execution
    last_err = None
    for attempt in range(3):
        try:
            res = run_bass_kernel_spmd(nc, in_maps, core_ids=list(range(B)))
            break
        except Exception as e:  # noqa: BLE001
            last_err = e
            time.sleep(2.0)
    else:
        raise last_err
    out = np.stack(
        [res.results[b]["out"].reshape(C, 64, 64) for b in range(B)]
    ).astype(np.float32)
    return out
